# revision 38
# baseline (speedup 1.0000x reference)
"""Trainium2 Bass kernel: GroupNorm + single-head self-attention + residual.

Reference computation (B=4, C=256, L=4096, GROUPS=8):
    xn   = GroupNorm(x) * gn_w + gn_b
    qkv  = w_qkv @ xn + b_qkv          # 1x1 conv
    attn = softmax(q^T k / sqrt(C))
    out  = w_out @ (attn @ v) + b_out + x

Sharding: 8 cores = (batch b, query-half h). Each core computes GN stats and
k/v over all L (redundant with its sibling core, but cheap), and q/attention/
output projection only for its 2048 query positions. No collectives.

v2 design (vs the IC=512 fp8 baseline; sim 98us vs 123, HW ~30% faster in
same-window A/B — note the shared axon terminal's absolute timings drift
~2x between sessions, so only same-session comparisons are meaningful):
  - IC=1024 (2 query chunks of the 2048 queries). Scores for j-tile j are 2
    MMs (i-halves) sharing one k-stationary; av per pair is 2 MMs sharing a
    v-pair stationary (the duplicate Ldweights is still emitted — bass has
    no reuse form — but it is adjacent, so the PE weight-load pipelining /
    reorder window can overlap it with the sibling MM's stream).
  - Cross-engine sem latency (~0.2-0.9us visible in TimelineSim) must be
    hidden by PSUM slot depth. PSUM (8 banks exactly): tag "sc" 4 bufs x
    [P,512]f32 1-bank scores tiles (a 2-buf version serialized the whole
    loop at ~2.1us/j-tile); tag "av" 1 buf x [P,1024] (av accumulates one
    ct at a time: ct0 one-pair-behind in-loop, ct1 as a pass over the live
    ex tiles, carried into the NEXT chunk's jp0-5 for interior chunks and
    into two sc slots for the last chunk); tag "sm" 1 buf x [P,2,512]: ONE
    allocation per chunk reused in-place (sums -> ps_B -> proj psum) via
    WAR subtile deps. IMPORTANT: a pool-tag allocation only sees readers
    already emitted, so sm/av0 are allocated lazily and all deferred work
    on a slot must be emitted before the tag's next allocation.
  - exp per (j, i-half): DVE takes h0, ACT h1 (the av half-MM then depends
    on exactly one exp engine); for act_h0 j's ACT takes both halves (load
    balance: DVE also carries recip/av8; + DVE tail drain before recip).
  - 1/Z is folded into av8 (av8 = e4m3(av * rec * AV_SCALE), AV_SCALE=16
    so attnout*16 sits in e4m3's NORMAL range — at 1/16 the values landed
    subnormal and rel-err tripled) and x + bout_eff is prefolded into xq,
    so the res epilogue is one scalar_tensor_tensor per half.
  - prologue: qkv psum rotates over 6 one-bank slots (sc x4 + idle sm/av);
    wqkvT ships as bf16 (fold target is e4m3 anyway; bias matmuls go
    bf16 x bf16); SWDGE queue carries only x8o/woT/xq (~1us generation
    cost per SWDGE dma_start) with the big xq last; small consts ride the
    sync queue after the weights.
  - recip writes f32r directly (no extra rounding copy); last chunk's tail
    is pipelined per i-half (av8(h) -> proj(h) -> res(h) -> DMA(h)).
Numerics follow the fp8 baseline: e4m3 x / folded weights, scoresT via DR
with k stationary, exp e^s/4 in e4m3 split ACT (exp table, bias -2ln2) /
DVE (Schraudolph u8 bits), wo8 host-prescaled x8, RES_SCALE=1/128 epilogue.
"""

import numpy as np
import ml_dtypes

import concourse.bass as bass
import concourse.mybir as mybir
from concourse import bacc
import concourse.tile as tile
from concourse.bass_utils import run_bass_kernel_spmd
from concourse.tile_rust import add_dep_helper

P = 128
C = 256
L = 4096
LH = 2048  # query positions per core
B = 4
N_CORES = 8
CT = C // P  # 2 c-tiles
JT = L // P  # 32 j-tiles
JPC = JT // 2  # 16 j-pairs
EPS = 1e-5

F32 = mybir.dt.float32
BF16 = mybir.dt.bfloat16
F32R = mybir.dt.float32r
F8 = mybir.dt.float8e4
U8 = mybir.dt.uint8
E4NP = ml_dtypes.float8_e4m3fn

AF = mybir.ActivationFunctionType
ALU = mybir.AluOpType
DR = mybir.MatmulPerfMode.DoubleRow

IC = 1024          # query-chunk width
NCHK = LH // IC    # 2
HW = 512           # i-half width (PSUM bank = 512 fp32)

# Softmax scale 1/sqrt(C) is applied inside exp (ACT scale= / DVE slope).
SM_SCALE = 1.0 / 16.0
# Schraudolph e4m3-bit exp: bits = round(EXP_A*s_raw + EXP_C); value ~= e^s/4.
EXP_A = 11.5415603 * SM_SCALE  # 8 * log2(e) / 16
EXP_C = 39.6435                # 8 * (7 - 0.0450) - 16
M2LN2 = -1.38629436            # ACT-path bias: exp(s/16 - 2ln2) = e^s / 4
# av8 = e4m3(av * rec * AV_SCALE) = attnout * AV_SCALE. attnout ~ N(0,1)
# (convex combo of v), so scale UP to sit in e4m3's normal range (max ~6
# sigma * 16 = 96 < 240); undone by RES_SCALE = 1/(AV_SCALE*WO_SCALE).
AV_SCALE = 16.0
WO_SCALE = 8.0          # host prescale of wo8
RES_SCALE = 1.0 / (AV_SCALE * WO_SCALE)  # = 1/128


def _r(ap):
    return ap.bitcast(F32R)


def build_nc(compile: bool = True, reps: int = 1, unroll: bool = False):
    nc = bacc.Bacc("TRN2")

    # --- I/O ----------------------------------------------------------------
    x8q_d = nc.declare_dram_parameter("x8q", [C, LH], F8, isOutput=False)
    x8o_d = nc.declare_dram_parameter("x8o", [C, LH], F8, isOutput=False)
    xq_d = nc.declare_dram_parameter("xq", [C, LH], F32, isOutput=False)
    wqkvT_d = nc.declare_dram_parameter("wqkvT", [C, 3 * C], BF16, isOutput=False)
    bqkv_d = nc.declare_dram_parameter("bqkv6", [P, 6], F32, isOutput=False)
    woutT_d = nc.declare_dram_parameter("woutT", [C, C], F32, isOutput=False)
    bout_d = nc.declare_dram_parameter("bout2", [P, CT], F32, isOutput=False)
    gnw_d = nc.declare_dram_parameter("gnw2", [P, CT], F32, isOutput=False)
    gnb_d = nc.declare_dram_parameter("gnb2", [P, CT], F32, isOutput=False)
    sel_d = nc.declare_dram_parameter("sel", [P, 4], F32, isOutput=False)
    selT_d = nc.declare_dram_parameter("selT", [4, P], F32, isOutput=False)
    wo8_d = nc.declare_dram_parameter("wo8", [C, C], F8, isOutput=False)
    oner_d = nc.declare_dram_parameter("ones_row", [1, P], F32, isOutput=False)
    out_d = nc.declare_dram_parameter("out", [C, LH], F32, isOutput=True)

    with tile.TileContext(nc) as tc, \
         tc.tile_pool(name="const", bufs=1) as const, \
         tc.tile_pool(name="xbuf", bufs=1) as xbuf, \
         tc.tile_pool(name="qkv", bufs=1) as qkvp, \
         tc.tile_pool(name="work", bufs=3) as work, \
         tc.tile_pool(name="res", bufs=2) as resp, \
         tc.tile_pool(name="exppool", bufs=20) as exppool, \
         tc.tile_pool(name="psum", bufs=1, space="PSUM") as psum:

        def emit_body():
            # --- DMA in -----------------------------------------------------
            x8q = xbuf.tile([P, CT, LH], F8)
            x8o = xbuf.tile([P, CT, LH], F8)
            x8q3 = x8q_d[:].rearrange("(t p) l -> p t l", p=P)
            x8o3 = x8o_d[:].rearrange("(t p) l -> p t l", p=P)
            NCH = 4
            CW = LH // NCH
            # stats-critical x8q chunks on the sync/HWDGE queue
            for n in range(NCH):
                for t in range(CT):
                    sl = slice(n * CW, (n + 1) * CW)
                    nc.sync.dma_start(x8q[:, t, sl], x8q3[:, t, sl])
            # SWDGE queue: x8o first (stats tail — each SWDGE dma_start pays
            # ~1us generation, so keep this queue short), then woT, then the
            # big fp32 residual xq (not needed until phase2)
            for t in range(CT):
                nc.gpsimd.dma_start(x8o[:, t, :], x8o3[:, t, :])
            woT = const.tile([P, CT, C], F32R)
            nc.gpsimd.dma_start(woT, _r(woutT_d[:].rearrange("(t p) o -> p t o", p=P)))
            xq = xbuf.tile([P, CT, LH], F32)
            xq3 = xq_d[:].rearrange("(t p) l -> p t l", p=P)
            for t in range(CT):
                nc.gpsimd.dma_start(xq[:, t, :], xq3[:, t, :])

            # Preload the exp ACT table set while x streams in.
            warm = work.tile([4, 1], F32, tag="warm")
            nc.vector.memset(warm, 0.0)
            nc.scalar.activation(warm, warm, AF.Exp)

            # weights/consts on sync queue after x8q
            wT = const.tile([P, CT, 3 * C], BF16)
            nc.sync.dma_start(wT, wqkvT_d[:].rearrange("(t p) o -> p t o", p=P))
            bqkv = const.tile([P, 6], F32)
            nc.sync.dma_start(bqkv, bqkv_d[:])
            bout = const.tile([P, CT], F32)
            nc.sync.dma_start(bout, bout_d[:])
            gnw = const.tile([P, CT], F32)
            nc.sync.dma_start(gnw, gnw_d[:])
            gnb = const.tile([P, CT], F32)
            nc.sync.dma_start(gnb, gnb_d[:])
            # tiny constants ride the sync queue (HWDGE) after the weights
            sel = const.tile([P, 4], F32R)
            nc.sync.dma_start(sel, _r(sel_d[:]))
            selT = const.tile([4, P], F32R)
            nc.sync.dma_start(selT, _r(selT_d[:]))
            wo8 = const.tile([P, CT, C], F8)
            nc.sync.dma_start(wo8, wo8_d[:].rearrange("(t p) o -> p t o", p=P))
            ones_row = const.tile([1, P], F32R)
            nc.sync.dma_start(ones_row, _r(oner_d[:]))
            ones8 = const.tile([P, 2, 16], F8)
            nc.vector.memset(ones8.bitcast(U8), 0x38)  # 1.0 in e4m3
            m2ln2 = const.tile([P, 1], F32)
            nc.vector.memset(m2ln2, M2LN2)

            # --- GroupNorm stats (from e4m3 x) ------------------------------
            SW = 512
            nst = LH // SW
            AW = 1536
            NACT = 1
            stats = work.tile([P, CT, nst + 1, 6], F32, tag="bnstats")
            s_acc = work.tile([P, CT, NACT, 2], F32, tag="sacc")
            for t in range(CT):
                for n in range(nst):
                    sl = slice(n * SW, (n + 1) * SW)
                    nc.vector.bn_stats(stats[:, t, n, :], x8q[:, t, sl])
                nc.vector.bn_stats(stats[:, t, nst, :], x8o[:, t, 0:SW])
                sl = slice(SW, SW + AW)
                scr = work.tile([P, AW], F32, tag="actscr")
                nc.scalar.activation(scr, x8o[:, t, sl], AF.Identity,
                                     accum_out=s_acc[:, t, 0, 0:1])
                scr2 = work.tile([P, AW], F32, tag="actscr2")
                nc.scalar.activation(scr2, x8o[:, t, sl], AF.Square,
                                     accum_out=s_acc[:, t, 0, 1:2])
            # HAM warm-up burst while the PE is otherwise idle during stats.
            ps_w = psum.tile([4, 512], F32, tag="av")
            for wi in range(16):
                nc.tensor.matmul(ps_w, ones8[:, :, 0:4],
                                 x8q[:, :, (wi % 4) * 512:(wi % 4 + 1) * 512],
                                 start=(wi == 0), stop=(wi == 15), perf_mode=DR)
            wsum = work.tile([4, 1], F32, tag="wsum")
            nc.vector.tensor_copy(wsum, ps_w[:, 0:1])
            wone = work.tile([4, 1], F32, tag="wone")
            nc.vector.tensor_scalar(wone, wsum, 0.0, 1.0, ALU.mult, ALU.add)

            W_BN = float(nst + 1) / (2 * nst)
            INV_N = 1.0 / (2 * nst * SW)
            rs4 = work.tile([P, CT, 4], F32, tag="rs")
            for t in range(CT):
                nc.vector.bn_aggr(rs4[:, t, 0:2], stats[:, t, :, :])
            m2 = work.tile([P, CT], F32, tag="e2bn")
            nc.vector.tensor_tensor(m2, rs4[:, :, 0], rs4[:, :, 0], ALU.mult)
            nc.vector.tensor_tensor(rs4[:, :, 1], rs4[:, :, 1], m2, ALU.add)
            nc.vector.reduce_sum(rs4[:, :, 2:4],
                                 s_acc.rearrange("p t n k -> p t k n"),
                                 axis=mybir.AxisListType.X)

            ps_g = psum.tile([4, 4 * CT], F32, tag="sm")
            nc.tensor.matmul(ps_g, sel.bitcast(F32),
                             rs4.rearrange("p t k -> p (t k)").bitcast(F32),
                             start=True, stop=True)
            g4 = work.tile([4, CT, 4], F32, tag="g4")
            nc.vector.tensor_copy(g4, ps_g.rearrange("j (t k) -> j t k", k=4))
            pk = work.tile([4, 2 * CT], F32R, tag="pk")
            pk3 = pk.rearrange("j (a t) -> j a t", a=2)
            gmt = work.tile([4, CT], F32, tag="gsb")
            nc.vector.scalar_tensor_tensor(gmt, g4[:, :, 2], INV_N / W_BN,
                                           g4[:, :, 0], ALU.mult, ALU.add)
            nc.vector.tensor_scalar_mul(pk3[:, 1, :], gmt, W_BN / 32.0)
            e2t = work.tile([4, CT], F32, tag="e2t")
            nc.vector.scalar_tensor_tensor(e2t, g4[:, :, 3], INV_N / W_BN,
                                           g4[:, :, 1], ALU.mult, ALU.add)
            vg = work.tile([4, CT], F32, tag="vg")
            nc.vector.tensor_tensor(vg, pk3[:, 1, :].bitcast(F32),
                                    pk3[:, 1, :].bitcast(F32), ALU.mult)
            nc.vector.scalar_tensor_tensor(vg, e2t, W_BN / 32.0, vg,
                                           ALU.mult, ALU.subtract)
            nc.vector.tensor_scalar_add(vg, vg, EPS)
            nwy = work.tile([4, CT], F32, tag="nwy")
            nc.vector.tensor_scalar(nwy, vg, -0.5, 1.5, ALU.mult, ALU.add)
            nwt = work.tile([4, CT], F32, tag="nwt")
            for _ in range(1):
                nc.vector.tensor_tensor(nwt, nwy, nwy, ALU.mult)
                nc.vector.tensor_tensor(nwt, vg, nwt, ALU.mult)
                nc.vector.tensor_scalar(nwt, nwt, -0.5, 1.5, ALU.mult, ALU.add)
                nc.vector.tensor_tensor(nwy, nwy, nwt, ALU.mult)
            nc.vector.tensor_scalar_mul(pk3[:, 0, :], nwy, warm[:, 0:1])
            nc.vector.tensor_scalar_mul(pk3[:, 0, :], pk3[:, 0, :].bitcast(F32),
                                        wone)

            ps_bc = psum.tile([P, 2 * CT], F32, tag="sm")
            nc.tensor.matmul(ps_bc, selT.bitcast(F32), pk.bitcast(F32),
                             start=True, stop=True)
            gb3 = ps_bc.rearrange("p (a t) -> p a t", a=2)

            scale_c = work.tile([P, CT], F32, tag="scale_c")
            nc.vector.tensor_tensor(scale_c, gb3[:, 0, :], gnw, ALU.mult)

            wp8 = const.tile([P, CT, 3 * C], F8)
            for blk in range(3):
                bsl = slice(blk * C, (blk + 1) * C)
                nc.vector.tensor_scalar_mul(wp8[:, 0, bsl], wT[:, 0, bsl],
                                            scale_c[:, 0:1])
                nc.scalar.activation(wp8[:, 1, bsl], wT[:, 1, bsl],
                                     AF.Identity, scale=scale_c[:, 1:2])

            shift_c = work.tile([P, CT], F32R, tag="shift_c")
            nc.vector.tensor_tensor(shift_c, gb3[:, 1, :], scale_c, ALU.mult)
            nc.vector.tensor_tensor(shift_c, gnb, shift_c, ALU.subtract)

            shift_cb = work.tile([P, CT], BF16, tag="shift_cb")
            with nc.allow_low_precision(reason="bf16 bias matmul"):
                nc.vector.tensor_copy(shift_cb, shift_c.bitcast(F32))
            bias_eff = const.tile([P, 6], F32R)
            ps_b6 = psum.tile([P, 6], F32, tag="sm")
            for mt in range(6):
                for t in range(CT):
                    nc.tensor.matmul(ps_b6[:, mt : mt + 1],
                                     wT[:, t, mt * P : (mt + 1) * P],
                                     shift_cb[:, t : t + 1],
                                     start=(t == 0), stop=(t == CT - 1))
            nc.vector.tensor_tensor(bias_eff, ps_b6, bqkv, ALU.add)

            bout_eff = const.tile([P, CT], F32)
            ps_b2 = psum.tile([P, CT], F32, tag="sm")
            for mt in range(CT):
                for t in range(CT):
                    nc.tensor.matmul(ps_b2[:, mt : mt + 1],
                                     woT[:, t, mt * P : (mt + 1) * P].bitcast(F32),
                                     bias_eff[:, 4 + t : 5 + t].bitcast(F32),
                                     start=(t == 0), stop=(t == CT - 1))
            nc.vector.tensor_tensor(bout_eff, ps_b2, bout, ALU.add)

            # --- q, k, v projections (DoubleRow fp8) ------------------------
            # uniform 1-bank psum tiles rotating across 6 slots (sc x4 + the
            # sm/av slots, idle until attention) so the MM->evac sem round
            # trip pipelines deeply
            qkv_tags = [("sc", 4), ("sc", 4), ("sm", 1),
                        ("sc", 4), ("sc", 4), ("av", 1)]
            qkv_i = [0]

            def qkv_ps(shape):
                tag, bufs = qkv_tags[qkv_i[0] % len(qkv_tags)]
                qkv_i[0] += 1
                return psum.tile(shape, F32, tag=tag, bufs=bufs, name="ps_qkv")

            # q8[p, t, i]: one [P,512] psum tile + 512-wide evac per n-chunk
            q8 = qkvp.tile([P, CT, LH], F8)
            qi = 0
            for mt in range(CT):
                for n in range(LH // HW):
                    ps_q = qkv_ps([P, HW])
                    sl = slice(n * HW, (n + 1) * HW)
                    nc.tensor.matmul(ps_q,
                                     wp8[:, :, mt * P : (mt + 1) * P],
                                     x8q[:, :, sl], start=True, stop=True,
                                     perf_mode=DR)
                    if qi % 2 == 0:
                        nc.scalar.activation(q8[:, mt, sl], ps_q, AF.Identity,
                                             bias=bias_eff[:, mt : mt + 1].bitcast(F32))
                    else:
                        nc.vector.tensor_scalar(q8[:, mt, sl], ps_q,
                                                bias_eff[:, mt : mt + 1].bitcast(F32),
                                                None, ALU.add)
                    qi += 1

            # k8: [c_out, j] over both halves (own half first = j order)
            k8 = qkvp.tile([P, CT, L], F8)
            for mt in range(CT):
                for hh, xsrc in enumerate((x8q, x8o)):
                    for n in range(LH // HW):
                        ps_k = qkv_ps([P, HW])
                        sl = slice(n * HW, (n + 1) * HW)
                        nc.tensor.matmul(
                            ps_k,
                            wp8[:, :, (2 + mt) * P : (3 + mt) * P],
                            xsrc[:, :, sl], start=True, stop=True,
                            perf_mode=DR)
                        osl = slice(hh * LH + n * HW, hh * LH + (n + 1) * HW)
                        if qi % 2 == 0:
                            nc.scalar.activation(k8[:, mt, osl], ps_k, AF.Identity,
                                                 bias=bias_eff[:, 2 + mt : 3 + mt].bitcast(F32))
                        else:
                            nc.vector.tensor_scalar(k8[:, mt, osl], ps_k,
                                                    bias_eff[:, 2 + mt : 3 + mt].bitcast(F32),
                                                    None, ALU.add)
                        qi += 1

            # v8 transposed: [j, c]; two jb tiles share one psum bank
            v8 = qkvp.tile([P, JT, C], F8)
            for jb2 in range(JT // 2):
                ps_v = qkv_ps([P, 2, C])
                for u in range(2):
                    jb = 2 * jb2 + u
                    xsrc = x8q if jb < JT // 2 else x8o
                    off = (jb % (JT // 2)) * P
                    nc.tensor.matmul(ps_v[:, u, :], xsrc[:, :, off : off + P],
                                     wp8[:, :, 2 * C : 3 * C], start=True,
                                     stop=True, perf_mode=DR)
                if jb2 % 2 == 0:
                    nc.scalar.activation(v8[:, 2 * jb2 : 2 * jb2 + 2, :], ps_v,
                                         AF.Copy, bias=0.0)
                else:
                    nc.vector.tensor_copy(v8[:, 2 * jb2 : 2 * jb2 + 2, :], ps_v)

            # xr = x + bout_eff prefold (residual + output bias, in place)
            nc.vector.tensor_scalar(xq[:, 0, :], xq[:, 0, :],
                                    bout_eff[:, 0:1], None, ALU.add)
            nc.scalar.activation(xq[:, 1, :], xq[:, 1, :], AF.Identity,
                                 bias=bout_eff[:, 1:2])

            # --- attention --------------------------------------------------
            out3 = out_d[:].rearrange("(t p) l -> p t l", p=P)

            state = {}  # carry between chunks: c0 tail work emitted inside c1

            def emit_scores_pair(ch, jp, act_h0, prev_h1=None):
                """scores for one j-pair into 4 single-bank psum tiles; exp
                per (j, half): DVE takes h0, ACT h1 (h-aligned with the av
                half MMs, so each av MM waits on exactly one exp engine).
                For (jp, jj) in act_h0, ACT does both halves (load balance /
                DVE tail drain)."""
                ex = exppool.tile([P, 2, IC], F8, tag="exp")
                h1_ops = []
                for jj in range(2):
                    j = 2 * jp + jj
                    for h in range(2):
                        ps_sh = psum.tile([P, HW], F32, tag="sc", bufs=4,
                                          name="ps_sh")
                        isl = slice(ch * IC + h * HW, ch * IC + (h + 1) * HW)
                        mm = nc.tensor.matmul(ps_sh,
                                              k8[:, :, j * P : (j + 1) * P],
                                              q8[:, :, isl], start=True,
                                              stop=True, perf_mode=DR)
                        # glue the j's two MMs: h0's slot frees on the prev
                        # pair's DVE exp, h1's on its ACT exp — equalize
                        # readiness so the shared k stationary isn't reloaded
                        if h == 0 and prev_h1 and jj < len(prev_h1):
                            add_dep_helper(mm.ins, prev_h1[jj].ins, True,
                                           "glue scores pair")
                        osl = slice(h * HW, (h + 1) * HW)
                        if h == 0 and (jp, jj) not in act_h0:
                            nc.vector.tensor_scalar(
                                ex[:, jj, osl].bitcast(U8), ps_sh,
                                EXP_A, EXP_C, ALU.mult, ALU.add)
                        else:
                            op = nc.scalar.activation(ex[:, jj, osl], ps_sh,
                                                      AF.Exp, bias=m2ln2,
                                                      scale=SM_SCALE)
                            if h == 1:
                                h1_ops.append(op)
                return ex, h1_ops

            def emit_av(avh, ct, jp, ex, h1_ops=None, nstop=JPC - 1):
                """avh: list of two [P, HW] psum APs (i-halves)."""
                mm0 = None
                for h in range(2):
                    mm = nc.tensor.matmul(avh[h],
                                          v8[:, 2 * jp : 2 * jp + 2,
                                             ct * P : (ct + 1) * P],
                                          ex[:, :, h * HW : (h + 1) * HW],
                                          start=(jp == 0), stop=(jp == nstop),
                                          perf_mode=DR)
                    if h == 0:
                        mm0 = mm
                # glue the pair: the h0 MM is ready once DVE's exp lands but
                # h1 waits on ACT; without these deps the readiness-driven
                # scheduler inserts scores MMs between the two, forcing the
                # shared v-pair stationary to reload (~213ns/pair on HW).
                if h1_ops:
                    for op in h1_ops:
                        add_dep_helper(mm0.ins, op.ins, True, "glue av pair")

            def emit_sums(sm, exs, lo, hi, last):
                for i in range(lo, hi):
                    for k in range(2):
                        nc.tensor.matmul(sm[0:1, k, :], ones8[:, :, 0:1],
                                         exs[i][:, :, k * HW : (k + 1) * HW],
                                         start=(i == 0 and k == 0),
                                         stop=(last and i == hi - 1 and k == 1),
                                         perf_mode=DR)

            def emit_phase1(ch, sm, av0, emit_ct0=True):
                """recip -> B_sb broadcast -> av8 ct0. DVE + ACT + 2 tiny MMs."""
                rec = work.tile([1, 2, HW], F32R, tag="rec")
                with nc.allow_low_precision(reason="f32r for bcast matmul"):
                    nc.vector.reciprocal(rec, sm[0:1, :, :])
                ps_B = sm  # reuse the sums banks (WAR via tile deps)
                for k in range(2):
                    nc.tensor.matmul(ps_B[:, k, :], ones_row,
                                     rec[:, k, :], start=True, stop=True)
                B_sb = work.tile([P, 2, HW], F32, tag="bsb")
                nc.scalar.activation(B_sb[:, 0, :], ps_B[:, 0, :], AF.Copy,
                                     bias=0.0)
                nc.scalar.activation(B_sb[:, 1, :], ps_B[:, 1, :], AF.Copy,
                                     bias=0.0)
                av8 = resp.tile([P, CT, IC], F8, tag="av8")
                if emit_ct0:
                    for h in range(2):
                        emit_av8_h(av8, 0, h, av0[h], B_sb)
                return B_sb, av8

            def emit_av8_h(av8, ct, h, src_ap, B_sb):
                nc.vector.scalar_tensor_tensor(
                    av8[:, ct, h * HW : (h + 1) * HW], src_ap, AV_SCALE,
                    B_sb[:, h, :], ALU.mult, ALU.mult)

            def emit_av8_ct1(av1h, B_sb, av8):
                """av1h: list of two [P, HW] psum APs (may be separate tiles)."""
                for h in range(2):
                    nc.vector.scalar_tensor_tensor(
                        av8[:, 1, h * HW : (h + 1) * HW], av1h[h], AV_SCALE,
                        B_sb[:, h, :], ALU.mult, ALU.mult)

            def emit_phase2(ch, sm, av8, half, last=False):
                """proj MMs into the sm banks + single res stt + DMA out."""
                ssl = slice(half * HW, (half + 1) * HW)
                isl = slice(ch * IC + half * HW, ch * IC + (half + 1) * HW)
                ps_o = sm  # [P, 2(mt), HW] view of the sm tile
                for mt in range(CT):
                    nc.tensor.matmul(ps_o[:, mt, :],
                                     wo8[:, :, mt * P : (mt + 1) * P],
                                     av8[:, :, ssl], start=True, stop=True,
                                     perf_mode=DR)
                res = resp.tile([P, CT, HW], F32, tag="res")
                nc.vector.scalar_tensor_tensor(res, ps_o, RES_SCALE,
                                               xq[:, :, isl], ALU.mult, ALU.add)
                nc.sync.dma_start(out3[:, :, isl], res)

            for ch in range(NCHK):
                last = ch == NCHK - 1
                carried = "avct1" in state
                # j's where ACT takes both exp halves: load balance (DVE also
                # carries recip/av8) and, on the last chunk, DVE tail drain
                if last:
                    act_h0 = {(2, 1), (5, 1), (8, 1), (15, 0), (15, 1)}
                else:
                    act_h0 = {(2, 1), (5, 1), (8, 1), (11, 1), (14, 1)}

                # sm and av0 are allocated LAZILY: pool-slot WAR deps only
                # cover already-emitted readers, so the alloc must come after
                # the previous chunk's users of the slot are all emitted.
                hold = {"sm": None, "av0": None}

                def get_sm():
                    if hold["sm"] is None:
                        hold["sm"] = psum.tile([P, 2, HW], F32, tag="sm",
                                               name="sm")
                    return hold["sm"]

                def get_av0():
                    if hold["av0"] is None:
                        t = psum.tile([P, IC], F32, tag="av", name="av0")
                        v = t.rearrange("p (a b) -> p a b", a=2)
                        hold["av0"] = [v[:, 0, :], v[:, 1, :]]
                    return hold["av0"]

                exs = []
                exops = []
                av_done = 0
                for jp in range(JPC):
                    prev_h1 = exops[-1] if exops else state.get("lasth1")
                    ex, h1_ops = emit_scores_pair(ch, jp, act_h0, prev_h1)
                    exs.append(ex)
                    exops.append(h1_ops)
                    # carried tail work from the previous chunk, spread over
                    # 6 pairs so the PE doesn't burst at the chunk boundary
                    # (a 4-pair spread left DVE/ACT waiting on scores)
                    if jp <= 5 and carried:
                        pav1, pexs = state["avct1"]
                        lo, hi = [(0, 3), (3, 6), (6, 9), (9, 12),
                                  (12, 14), (14, 16)][jp]
                        for q in range(lo, hi):
                            emit_av(pav1, 1, q, pexs[q])
                        if jp == 5:
                            emit_av8_ct1(pav1, *state["p1"])
                            del state["avct1"]
                    if jp == 6 and "p2" in state:
                        pch, psm, pav8 = state["p2"]
                        emit_phase2(pch, psm, pav8, 0)
                        emit_phase2(pch, psm, pav8, 1)
                        del state["p2"]
                    # av ct0, one pair behind; deferred past the carried work
                    # (av0 must not be allocated before the previous chunk's
                    # av-ct1 consumers are emitted), catch-up rate-limited
                    if not (carried and jp < 6):
                        quota = 3 if carried else JPC
                        n = 0
                        while av_done < jp and n < quota:
                            emit_av(get_av0(), 0, av_done, exs[av_done],
                                    exops[av_done])
                            av_done += 1
                            n += 1
                    if jp == 7:
                        emit_sums(get_sm(), exs, 0, 6, False)
                    if jp == 13:
                        emit_sums(get_sm(), exs, 6, 12, False)
                while av_done < JPC:
                    emit_av(get_av0(), 0, av_done, exs[av_done],
                            exops[av_done])
                    av_done += 1
                emit_sums(get_sm(), exs, 12, JPC, True)

                state["lasth1"] = exops[-1]
                if not last:
                    B_sb, av8 = emit_phase1(ch, get_sm(), get_av0())
                    av1 = psum.tile([P, IC], F32, tag="av", name="av1")
                    av1v = av1.rearrange("p (a b) -> p a b", a=2)
                    state["avct1"] = ([av1v[:, 0, :], av1v[:, 1, :]], exs)
                    state["p1"] = (B_sb, av8)
                    state["p2"] = (ch, get_sm(), av8)
                else:
                    # last chunk: av ct1 accumulates in two scores slots so it
                    # overlaps the recip/B_sb chain instead of serializing
                    # behind av8 ct0's read of the av slot.
                    av1h = [psum.tile([P, HW], F32, tag="sc", name="av1h",
                                      bufs=4) for _ in range(2)]
                    for q in range(JPC):
                        emit_av(av1h, 1, q, exs[q], exops[q])
                    B_sb, av8 = emit_phase1(ch, get_sm(), get_av0(),
                                            emit_ct0=False)
                    # tail pipelined per i-half: av8(h) -> proj/res/DMA(h)
                    for half in range(2):
                        emit_av8_h(av8, 0, half, get_av0()[half], B_sb)
                        emit_av8_h(av8, 1, half, av1h[half], B_sb)
                        emit_phase2(ch, get_sm(), av8, half, last=True)

        if reps > 1 and unroll:
            for _ in range(reps):
                emit_body()
        elif reps > 1:
            with tc.For_i(0, reps, 1):
                emit_body()
        else:
            emit_body()

    if compile:
        nc.compile()
    return nc


def make_host_inputs(x, gn_w, gn_b, w_qkv, b_qkv, w_out, b_out):
    """Shared (weight) arrays + per-core (x8q, x8o, xq) shards."""
    wqkvT = np.ascontiguousarray(w_qkv.T).astype(ml_dtypes.bfloat16)
    bqkv6 = np.ascontiguousarray(b_qkv.astype(np.float32).reshape(6, P).T)
    woutT = np.ascontiguousarray(w_out.T).astype(np.float32)
    wo8 = (woutT * WO_SCALE).astype(E4NP)
    bout2 = np.ascontiguousarray(b_out.astype(np.float32).reshape(CT, P).T)
    gnw2 = np.ascontiguousarray(gn_w.astype(np.float32).reshape(CT, P).T)
    gnb2 = np.ascontiguousarray(gn_b.astype(np.float32).reshape(CT, P).T)
    pidx = np.arange(P)
    sel = (pidx[:, None] // 32 == np.arange(4)[None, :]).astype(np.float32)
    selT = np.ascontiguousarray(sel.T)

    ones_row = np.ones((1, P), np.float32)
    shared = dict(wqkvT=wqkvT, bqkv6=bqkv6, woutT=woutT, wo8=wo8, bout2=bout2,
                  gnw2=gnw2, gnb2=gnb2, sel=sel, selT=selT, ones_row=ones_row)

    in_maps = []
    for core in range(N_CORES):
        b, h = divmod(core, 2)
        own = slice(h * LH, (h + 1) * LH)
        oth = slice((1 - h) * LH, (2 - h) * LH)
        m = dict(shared)
        xq_f32 = np.ascontiguousarray(x[b][:, own]).astype(np.float32)
        xo_f32 = np.ascontiguousarray(x[b][:, oth]).astype(np.float32)
        m["xq"] = xq_f32
        m["x8q"] = xq_f32.astype(E4NP)
        m["x8o"] = xo_f32.astype(E4NP)
        in_maps.append(m)
    return in_maps


_NC = None


def kernel(x, gn_w, gn_b, w_qkv, b_qkv, w_out, b_out, _trace=False, **_kw):
    global _NC
    x = np.asarray(x)
    if _NC is None:
        _NC = build_nc()
    in_maps = make_host_inputs(np.asarray(x), np.asarray(gn_w), np.asarray(gn_b),
                               np.asarray(w_qkv), np.asarray(b_qkv),
                               np.asarray(w_out), np.asarray(b_out))
    kw = {}
    if _trace:
        kw = dict(trace=True)
    br = run_bass_kernel_spmd(_NC, in_maps, list(range(N_CORES)), **kw)
    out = np.empty((B, C, L), np.float32)
    for core in range(N_CORES):
        b, h = divmod(core, 2)
        out[b][:, h * LH : (h + 1) * LH] = br.results[core]["out"]
    if _trace:
        return out, br
    return out


# revision 43
# speedup vs baseline: 1.2361x; 1.2361x over previous
"""Trainium2 Bass kernel: GroupNorm + single-head self-attention + residual.

Reference computation (B=4, C=256, L=4096, GROUPS=8):
    xn   = GroupNorm(x) * gn_w + gn_b
    qkv  = w_qkv @ xn + b_qkv          # 1x1 conv
    attn = softmax(q^T k / sqrt(C))
    out  = w_out @ (attn @ v) + b_out + x

Sharding: 8 cores = (batch b, query-half h). Each core computes GN stats and
k/v over all L (redundant with its sibling core, but cheap), and q/attention/
output projection only for its 2048 query positions. No collectives.

v2 design (vs the IC=512 fp8 baseline; sim 98us vs 123, HW ~30% faster in
same-window A/B — note the shared axon terminal's absolute timings drift
~2x between sessions, so only same-session comparisons are meaningful):
  - IC=1024 (2 query chunks of the 2048 queries). Scores for j-tile j are 2
    MMs (i-halves) sharing one k-stationary; av per pair is 2 MMs sharing a
    v-pair stationary (the duplicate Ldweights is still emitted — bass has
    no reuse form — but it is adjacent, so the PE weight-load pipelining /
    reorder window can overlap it with the sibling MM's stream).
  - Cross-engine sem latency (~0.2-0.9us visible in TimelineSim) must be
    hidden by PSUM slot depth. PSUM (8 banks exactly): tag "sc" 4 bufs x
    [P,512]f32 1-bank scores tiles (a 2-buf version serialized the whole
    loop at ~2.1us/j-tile); tag "av" 1 buf x [P,1024] (av accumulates one
    ct at a time: ct0 one-pair-behind in-loop, ct1 as a pass over the live
    ex tiles, carried into the NEXT chunk's jp0-3 for interior chunks and
    into two sc slots for the last chunk); tag "sm" 1 buf x [P,2,512]: ONE
    allocation per chunk reused in-place (sums -> ps_B -> proj psum) via
    WAR subtile deps. IMPORTANT: a pool-tag allocation only sees readers
    already emitted, so sm/av0 are allocated lazily and all deferred work
    on a slot must be emitted before the tag's next allocation.
  - exp per (j, i-half): DVE takes h0, ACT h1 (the av half-MM then depends
    on exactly one exp engine); for act_h0 j's ACT takes both halves (load
    balance: DVE also carries recip/av8; + DVE tail drain before recip).
  - 1/Z is folded into av8 (av8 = e4m3(av * rec * AV_SCALE), AV_SCALE=16
    so attnout*16 sits in e4m3's NORMAL range — at 1/16 the values landed
    subnormal and rel-err tripled) and x + bout_eff is prefolded into xq,
    so the res epilogue is one scalar_tensor_tensor per half.
  - prologue: qkv psum rotates over 6 one-bank slots (sc x4 + idle sm/av);
    wqkvT ships as bf16 (fold target is e4m3 anyway; bias matmuls go
    bf16 x bf16); SWDGE queue carries only x8o/woT/xq (~1us generation
    cost per SWDGE dma_start) with the big xq last; small consts ride the
    sync queue after the weights.
  - recip writes f32r directly (no extra rounding copy); last chunk's tail
    is pipelined per i-half (av8(h) -> proj(h) -> res(h) -> DMA(h)).
Numerics follow the fp8 baseline: e4m3 x / folded weights, scoresT via DR
with k stationary, exp e^s/4 in e4m3 split ACT (exp table, bias -2ln2) /
DVE (Schraudolph u8 bits), wo8 host-prescaled x8, RES_SCALE=1/128 epilogue.
"""

import numpy as np
import ml_dtypes

import concourse.bass as bass
import concourse.mybir as mybir
from concourse import bacc
import concourse.tile as tile
from concourse.bass_utils import run_bass_kernel_spmd
from concourse.tile_rust import add_dep_helper

P = 128
C = 256
L = 4096
LH = 2048  # query positions per core
B = 4
N_CORES = 8
CT = C // P  # 2 c-tiles
JT = L // P  # 32 j-tiles
JPC = JT // 2  # 16 j-pairs
EPS = 1e-5

F32 = mybir.dt.float32
BF16 = mybir.dt.bfloat16
F32R = mybir.dt.float32r
F8 = mybir.dt.float8e4
U8 = mybir.dt.uint8
E4NP = ml_dtypes.float8_e4m3fn

AF = mybir.ActivationFunctionType
ALU = mybir.AluOpType
DR = mybir.MatmulPerfMode.DoubleRow

IC = 1024          # query-chunk width
NCHK = LH // IC    # 2
HW = 512           # i-half width (PSUM bank = 512 fp32)

# Softmax scale 1/sqrt(C) is applied inside exp (ACT scale= / DVE slope).
SM_SCALE = 1.0 / 16.0
# Schraudolph e4m3-bit exp: bits = round(EXP_A*s_raw + EXP_C); value ~= e^s/4.
EXP_A = 11.5415603 * SM_SCALE  # 8 * log2(e) / 16
EXP_C = 39.6435                # 8 * (7 - 0.0450) - 16
M2LN2 = -1.38629436            # ACT-path bias: exp(s/16 - 2ln2) = e^s / 4
# av8 = e4m3(av * rec * AV_SCALE) = attnout * AV_SCALE. attnout ~ N(0,1)
# (convex combo of v), so scale UP to sit in e4m3's normal range (max ~6
# sigma * 16 = 96 < 240); undone by RES_SCALE = 1/(AV_SCALE*WO_SCALE).
AV_SCALE = 16.0
WO_SCALE = 8.0          # host prescale of wo8
RES_SCALE = 1.0 / (AV_SCALE * WO_SCALE)  # = 1/128


def _r(ap):
    return ap.bitcast(F32R)


def build_nc(compile: bool = True, reps: int = 1):
    nc = bacc.Bacc("TRN2")

    # --- I/O ----------------------------------------------------------------
    x8q_d = nc.declare_dram_parameter("x8q", [C, LH], F8, isOutput=False)
    x8o_d = nc.declare_dram_parameter("x8o", [C, LH], F8, isOutput=False)
    xq_d = nc.declare_dram_parameter("xq", [C, LH], F32, isOutput=False)
    wqkvT_d = nc.declare_dram_parameter("wqkvT", [C, 3 * C], BF16, isOutput=False)
    bqkv_d = nc.declare_dram_parameter("bqkv6", [P, 6], F32, isOutput=False)
    woutT_d = nc.declare_dram_parameter("woutT", [C, C], F32, isOutput=False)
    bout_d = nc.declare_dram_parameter("bout2", [P, CT], F32, isOutput=False)
    gnw_d = nc.declare_dram_parameter("gnw2", [P, CT], F32, isOutput=False)
    gnb_d = nc.declare_dram_parameter("gnb2", [P, CT], F32, isOutput=False)
    sel_d = nc.declare_dram_parameter("sel", [P, 4], F32, isOutput=False)
    selT_d = nc.declare_dram_parameter("selT", [4, P], F32, isOutput=False)
    wo8_d = nc.declare_dram_parameter("wo8", [C, C], F8, isOutput=False)
    oner_d = nc.declare_dram_parameter("ones_row", [1, P], F32, isOutput=False)
    out_d = nc.declare_dram_parameter("out", [C, LH], F32, isOutput=True)

    with tile.TileContext(nc) as tc, \
         tc.tile_pool(name="const", bufs=1) as const, \
         tc.tile_pool(name="xbuf", bufs=1) as xbuf, \
         tc.tile_pool(name="qkv", bufs=1) as qkvp, \
         tc.tile_pool(name="work", bufs=3) as work, \
         tc.tile_pool(name="res", bufs=2) as resp, \
         tc.tile_pool(name="exppool", bufs=20) as exppool, \
         tc.tile_pool(name="psum", bufs=1, space="PSUM") as psum:

        def emit_body():
            # --- DMA in -----------------------------------------------------
            x8q = xbuf.tile([P, CT, LH], F8)
            x8o = xbuf.tile([P, CT, LH], F8)
            x8q3 = x8q_d[:].rearrange("(t p) l -> p t l", p=P)
            x8o3 = x8o_d[:].rearrange("(t p) l -> p t l", p=P)
            NCH = 4
            CW = LH // NCH
            # stats-critical x8q chunks on the sync/HWDGE queue
            for n in range(NCH):
                for t in range(CT):
                    sl = slice(n * CW, (n + 1) * CW)
                    nc.sync.dma_start(x8q[:, t, sl], x8q3[:, t, sl])
            # SWDGE queue: x8o first (stats tail — each SWDGE dma_start pays
            # ~1us generation, so keep this queue short), then woT, then the
            # big fp32 residual xq (not needed until phase2)
            for t in range(CT):
                nc.gpsimd.dma_start(x8o[:, t, :], x8o3[:, t, :])
            woT = const.tile([P, CT, C], F32R)
            nc.gpsimd.dma_start(woT, _r(woutT_d[:].rearrange("(t p) o -> p t o", p=P)))
            xq = xbuf.tile([P, CT, LH], F32)
            xq3 = xq_d[:].rearrange("(t p) l -> p t l", p=P)
            for t in range(CT):
                nc.gpsimd.dma_start(xq[:, t, :], xq3[:, t, :])

            # Preload the exp ACT table set while x streams in.
            warm = work.tile([4, 1], F32, tag="warm")
            nc.vector.memset(warm, 0.0)
            nc.scalar.activation(warm, warm, AF.Exp)

            # weights/consts on sync queue after x8q
            wT = const.tile([P, CT, 3 * C], BF16)
            nc.sync.dma_start(wT, wqkvT_d[:].rearrange("(t p) o -> p t o", p=P))
            bqkv = const.tile([P, 6], F32)
            nc.sync.dma_start(bqkv, bqkv_d[:])
            bout = const.tile([P, CT], F32)
            nc.sync.dma_start(bout, bout_d[:])
            gnw = const.tile([P, CT], F32)
            nc.sync.dma_start(gnw, gnw_d[:])
            gnb = const.tile([P, CT], F32)
            nc.sync.dma_start(gnb, gnb_d[:])
            # tiny constants ride the sync queue (HWDGE) after the weights
            sel = const.tile([P, 4], F32R)
            nc.sync.dma_start(sel, _r(sel_d[:]))
            selT = const.tile([4, P], F32R)
            nc.sync.dma_start(selT, _r(selT_d[:]))
            wo8 = const.tile([P, CT, C], F8)
            nc.sync.dma_start(wo8, wo8_d[:].rearrange("(t p) o -> p t o", p=P))
            ones_row = const.tile([1, P], F32R)
            nc.sync.dma_start(ones_row, _r(oner_d[:]))
            ones8 = const.tile([P, 2, 16], F8)
            nc.vector.memset(ones8.bitcast(U8), 0x38)  # 1.0 in e4m3
            m2ln2 = const.tile([P, 1], F32)
            nc.vector.memset(m2ln2, M2LN2)

            # --- GroupNorm stats (from e4m3 x) ------------------------------
            SW = 512
            nst = LH // SW
            AW = 1536
            NACT = 1
            stats = work.tile([P, CT, nst + 1, 6], F32, tag="bnstats")
            s_acc = work.tile([P, CT, NACT, 2], F32, tag="sacc")
            for t in range(CT):
                for n in range(nst):
                    sl = slice(n * SW, (n + 1) * SW)
                    nc.vector.bn_stats(stats[:, t, n, :], x8q[:, t, sl])
                nc.vector.bn_stats(stats[:, t, nst, :], x8o[:, t, 0:SW])
                sl = slice(SW, SW + AW)
                scr = work.tile([P, AW], F32, tag="actscr")
                nc.scalar.activation(scr, x8o[:, t, sl], AF.Identity,
                                     accum_out=s_acc[:, t, 0, 0:1])
                scr2 = work.tile([P, AW], F32, tag="actscr2")
                nc.scalar.activation(scr2, x8o[:, t, sl], AF.Square,
                                     accum_out=s_acc[:, t, 0, 1:2])
            # HAM warm-up burst while the PE is otherwise idle during stats.
            ps_w = psum.tile([4, 512], F32, tag="av")
            for wi in range(16):
                nc.tensor.matmul(ps_w, ones8[:, :, 0:4],
                                 x8q[:, :, (wi % 4) * 512:(wi % 4 + 1) * 512],
                                 start=(wi == 0), stop=(wi == 15), perf_mode=DR)
            wsum = work.tile([4, 1], F32, tag="wsum")
            nc.vector.tensor_copy(wsum, ps_w[:, 0:1])
            wone = work.tile([4, 1], F32, tag="wone")
            nc.vector.tensor_scalar(wone, wsum, 0.0, 1.0, ALU.mult, ALU.add)

            W_BN = float(nst + 1) / (2 * nst)
            INV_N = 1.0 / (2 * nst * SW)
            rs4 = work.tile([P, CT, 4], F32, tag="rs")
            for t in range(CT):
                nc.vector.bn_aggr(rs4[:, t, 0:2], stats[:, t, :, :])
            m2 = work.tile([P, CT], F32, tag="e2bn")
            nc.vector.tensor_tensor(m2, rs4[:, :, 0], rs4[:, :, 0], ALU.mult)
            nc.vector.tensor_tensor(rs4[:, :, 1], rs4[:, :, 1], m2, ALU.add)
            nc.vector.reduce_sum(rs4[:, :, 2:4],
                                 s_acc.rearrange("p t n k -> p t k n"),
                                 axis=mybir.AxisListType.X)

            ps_g = psum.tile([4, 4 * CT], F32, tag="sm")
            nc.tensor.matmul(ps_g, sel.bitcast(F32),
                             rs4.rearrange("p t k -> p (t k)").bitcast(F32),
                             start=True, stop=True)
            g4 = work.tile([4, CT, 4], F32, tag="g4")
            nc.vector.tensor_copy(g4, ps_g.rearrange("j (t k) -> j t k", k=4))
            pk = work.tile([4, 2 * CT], F32R, tag="pk")
            pk3 = pk.rearrange("j (a t) -> j a t", a=2)
            gmt = work.tile([4, CT], F32, tag="gsb")
            nc.vector.scalar_tensor_tensor(gmt, g4[:, :, 2], INV_N / W_BN,
                                           g4[:, :, 0], ALU.mult, ALU.add)
            nc.vector.tensor_scalar_mul(pk3[:, 1, :], gmt, W_BN / 32.0)
            e2t = work.tile([4, CT], F32, tag="e2t")
            nc.vector.scalar_tensor_tensor(e2t, g4[:, :, 3], INV_N / W_BN,
                                           g4[:, :, 1], ALU.mult, ALU.add)
            vg = work.tile([4, CT], F32, tag="vg")
            nc.vector.tensor_tensor(vg, pk3[:, 1, :].bitcast(F32),
                                    pk3[:, 1, :].bitcast(F32), ALU.mult)
            nc.vector.scalar_tensor_tensor(vg, e2t, W_BN / 32.0, vg,
                                           ALU.mult, ALU.subtract)
            nc.vector.tensor_scalar_add(vg, vg, EPS)
            nwy = work.tile([4, CT], F32, tag="nwy")
            nc.vector.tensor_scalar(nwy, vg, -0.5, 1.5, ALU.mult, ALU.add)
            nwt = work.tile([4, CT], F32, tag="nwt")
            for _ in range(1):
                nc.vector.tensor_tensor(nwt, nwy, nwy, ALU.mult)
                nc.vector.tensor_tensor(nwt, vg, nwt, ALU.mult)
                nc.vector.tensor_scalar(nwt, nwt, -0.5, 1.5, ALU.mult, ALU.add)
                nc.vector.tensor_tensor(nwy, nwy, nwt, ALU.mult)
            nc.vector.tensor_scalar_mul(pk3[:, 0, :], nwy, warm[:, 0:1])
            nc.vector.tensor_scalar_mul(pk3[:, 0, :], pk3[:, 0, :].bitcast(F32),
                                        wone)

            ps_bc = psum.tile([P, 2 * CT], F32, tag="sm")
            nc.tensor.matmul(ps_bc, selT.bitcast(F32), pk.bitcast(F32),
                             start=True, stop=True)
            gb3 = ps_bc.rearrange("p (a t) -> p a t", a=2)

            scale_c = work.tile([P, CT], F32, tag="scale_c")
            nc.vector.tensor_tensor(scale_c, gb3[:, 0, :], gnw, ALU.mult)

            wp8 = const.tile([P, CT, 3 * C], F8)
            for blk in range(3):
                bsl = slice(blk * C, (blk + 1) * C)
                nc.vector.tensor_scalar_mul(wp8[:, 0, bsl], wT[:, 0, bsl],
                                            scale_c[:, 0:1])
                nc.scalar.activation(wp8[:, 1, bsl], wT[:, 1, bsl],
                                     AF.Identity, scale=scale_c[:, 1:2])

            shift_c = work.tile([P, CT], F32R, tag="shift_c")
            nc.vector.tensor_tensor(shift_c, gb3[:, 1, :], scale_c, ALU.mult)
            nc.vector.tensor_tensor(shift_c, gnb, shift_c, ALU.subtract)

            shift_cb = work.tile([P, CT], BF16, tag="shift_cb")
            with nc.allow_low_precision(reason="bf16 bias matmul"):
                nc.vector.tensor_copy(shift_cb, shift_c.bitcast(F32))
            bias_eff = const.tile([P, 6], F32R)
            ps_b6 = psum.tile([P, 6], F32, tag="sm")
            for mt in range(6):
                for t in range(CT):
                    nc.tensor.matmul(ps_b6[:, mt : mt + 1],
                                     wT[:, t, mt * P : (mt + 1) * P],
                                     shift_cb[:, t : t + 1],
                                     start=(t == 0), stop=(t == CT - 1))
            nc.vector.tensor_tensor(bias_eff, ps_b6, bqkv, ALU.add)

            bout_eff = const.tile([P, CT], F32)
            ps_b2 = psum.tile([P, CT], F32, tag="sm")
            for mt in range(CT):
                for t in range(CT):
                    nc.tensor.matmul(ps_b2[:, mt : mt + 1],
                                     woT[:, t, mt * P : (mt + 1) * P].bitcast(F32),
                                     bias_eff[:, 4 + t : 5 + t].bitcast(F32),
                                     start=(t == 0), stop=(t == CT - 1))
            nc.vector.tensor_tensor(bout_eff, ps_b2, bout, ALU.add)

            # --- q, k, v projections (DoubleRow fp8) ------------------------
            # uniform 1-bank psum tiles rotating across 6 slots (sc x4 + the
            # sm/av slots, idle until attention) so the MM->evac sem round
            # trip pipelines deeply
            qkv_tags = [("sc", 4), ("sc", 4), ("sm", 1),
                        ("sc", 4), ("sc", 4), ("av", 1)]
            qkv_i = [0]

            def qkv_ps(shape):
                tag, bufs = qkv_tags[qkv_i[0] % len(qkv_tags)]
                qkv_i[0] += 1
                return psum.tile(shape, F32, tag=tag, bufs=bufs, name="ps_qkv")

            # q8[p, t, i]: one [P,512] psum tile + 512-wide evac per n-chunk
            q8 = qkvp.tile([P, CT, LH], F8)
            qi = 0
            for mt in range(CT):
                for n in range(LH // HW):
                    ps_q = qkv_ps([P, HW])
                    sl = slice(n * HW, (n + 1) * HW)
                    nc.tensor.matmul(ps_q,
                                     wp8[:, :, mt * P : (mt + 1) * P],
                                     x8q[:, :, sl], start=True, stop=True,
                                     perf_mode=DR)
                    if qi % 2 == 0:
                        nc.scalar.activation(q8[:, mt, sl], ps_q, AF.Identity,
                                             bias=bias_eff[:, mt : mt + 1].bitcast(F32))
                    else:
                        nc.vector.tensor_scalar(q8[:, mt, sl], ps_q,
                                                bias_eff[:, mt : mt + 1].bitcast(F32),
                                                None, ALU.add)
                    qi += 1

            # k8: [c_out, j] over both halves (own half first = j order)
            k8 = qkvp.tile([P, CT, L], F8)
            for mt in range(CT):
                for hh, xsrc in enumerate((x8q, x8o)):
                    for n in range(LH // HW):
                        ps_k = qkv_ps([P, HW])
                        sl = slice(n * HW, (n + 1) * HW)
                        nc.tensor.matmul(
                            ps_k,
                            wp8[:, :, (2 + mt) * P : (3 + mt) * P],
                            xsrc[:, :, sl], start=True, stop=True,
                            perf_mode=DR)
                        osl = slice(hh * LH + n * HW, hh * LH + (n + 1) * HW)
                        if qi % 2 == 0:
                            nc.scalar.activation(k8[:, mt, osl], ps_k, AF.Identity,
                                                 bias=bias_eff[:, 2 + mt : 3 + mt].bitcast(F32))
                        else:
                            nc.vector.tensor_scalar(k8[:, mt, osl], ps_k,
                                                    bias_eff[:, 2 + mt : 3 + mt].bitcast(F32),
                                                    None, ALU.add)
                        qi += 1

            # v8 transposed: [j, c]; two jb tiles share one psum bank
            v8 = qkvp.tile([P, JT, C], F8)
            for jb2 in range(JT // 2):
                ps_v = qkv_ps([P, 2, C])
                for u in range(2):
                    jb = 2 * jb2 + u
                    xsrc = x8q if jb < JT // 2 else x8o
                    off = (jb % (JT // 2)) * P
                    nc.tensor.matmul(ps_v[:, u, :], xsrc[:, :, off : off + P],
                                     wp8[:, :, 2 * C : 3 * C], start=True,
                                     stop=True, perf_mode=DR)
                if jb2 % 2 == 0:
                    last_v_copy = nc.scalar.activation(
                        v8[:, 2 * jb2 : 2 * jb2 + 2, :], ps_v, AF.Copy,
                        bias=0.0)
                else:
                    last_v_copy = nc.vector.tensor_copy(
                        v8[:, 2 * jb2 : 2 * jb2 + 2, :], ps_v)

            # xr = x + bout_eff prefold (residual + output bias, in place)
            nc.vector.tensor_scalar(xq[:, 0, :], xq[:, 0, :],
                                    bout_eff[:, 0:1], None, ALU.add)
            nc.scalar.activation(xq[:, 1, :], xq[:, 1, :], AF.Identity,
                                 bias=bout_eff[:, 1:2])

            # --- attention --------------------------------------------------
            vdep = {"op": last_v_copy, "done": False}
            out3 = out_d[:].rearrange("(t p) l -> p t l", p=P)

            state = {}  # carry between chunks: c0 tail work emitted inside c1

            def emit_scores_pair(ch, jp, act_h0):
                """scores for one j-pair into 4 single-bank psum tiles; exp
                per (j, half): DVE takes h0, ACT h1 (h-aligned with the av
                half MMs, so each av MM waits on exactly one exp engine).
                For (jp, jj) in act_h0, ACT does both halves (load balance /
                DVE tail drain)."""
                ex = exppool.tile([P, 2, IC], F8, tag="exp")
                for jj in range(2):
                    j = 2 * jp + jj
                    for h in range(2):
                        ps_sh = psum.tile([P, HW], F32, tag="sc", bufs=4,
                                          name="ps_sh")
                        isl = slice(ch * IC + h * HW, ch * IC + (h + 1) * HW)
                        mm = nc.tensor.matmul(ps_sh,
                                              k8[:, :, j * P : (j + 1) * P],
                                              q8[:, :, isl], start=True,
                                              stop=True, perf_mode=DR)
                        if not vdep["done"]:
                            add_dep_helper(mm.ins, vdep["op"].ins, True,
                                           "observe v8 before attention")
                            vdep["done"] = True
                        osl = slice(h * HW, (h + 1) * HW)
                        if h == 0 and (jp, jj) not in act_h0:
                            nc.vector.tensor_scalar(
                                ex[:, jj, osl].bitcast(U8), ps_sh,
                                EXP_A, EXP_C, ALU.mult, ALU.add)
                        else:
                            nc.scalar.activation(ex[:, jj, osl], ps_sh,
                                                 AF.Exp, bias=m2ln2,
                                                 scale=SM_SCALE)
                return ex

            def emit_av(avh, ct, jp, ex, nstop=JPC - 1):
                """avh: list of two [P, HW] psum APs (i-halves)."""
                for h in range(2):
                    nc.tensor.matmul(avh[h],
                                     v8[:, 2 * jp : 2 * jp + 2,
                                        ct * P : (ct + 1) * P],
                                     ex[:, :, h * HW : (h + 1) * HW],
                                     start=(jp == 0), stop=(jp == nstop),
                                     perf_mode=DR)

            def emit_sums(sm, exs, lo, hi, last):
                for i in range(lo, hi):
                    for k in range(2):
                        nc.tensor.matmul(sm[0:1, k, :], ones8[:, :, 0:1],
                                         exs[i][:, :, k * HW : (k + 1) * HW],
                                         start=(i == 0 and k == 0),
                                         stop=(last and i == hi - 1 and k == 1),
                                         perf_mode=DR)

            def emit_phase1(ch, sm, av0, emit_ct0=True):
                """recip -> B_sb broadcast -> av8 ct0. DVE + ACT + 2 tiny MMs."""
                rec = work.tile([1, 2, HW], F32R, tag="rec")
                with nc.allow_low_precision(reason="f32r for bcast matmul"):
                    nc.vector.reciprocal(rec, sm[0:1, :, :])
                ps_B = sm  # reuse the sums banks (WAR via tile deps)
                for k in range(2):
                    nc.tensor.matmul(ps_B[:, k, :], ones_row,
                                     rec[:, k, :], start=True, stop=True)
                B_sb = work.tile([P, 2, HW], F32, tag="bsb")
                nc.scalar.activation(B_sb[:, 0, :], ps_B[:, 0, :], AF.Copy,
                                     bias=0.0)
                nc.scalar.activation(B_sb[:, 1, :], ps_B[:, 1, :], AF.Copy,
                                     bias=0.0)
                av8 = resp.tile([P, CT, IC], F8, tag="av8")
                if emit_ct0:
                    for h in range(2):
                        emit_av8_h(av8, 0, h, av0[h], B_sb)
                return B_sb, av8

            def emit_av8_h(av8, ct, h, src_ap, B_sb):
                nc.vector.scalar_tensor_tensor(
                    av8[:, ct, h * HW : (h + 1) * HW], src_ap, AV_SCALE,
                    B_sb[:, h, :], ALU.mult, ALU.mult)

            def emit_av8_ct1(av1h, B_sb, av8):
                """av1h: list of two [P, HW] psum APs (may be separate tiles)."""
                for h in range(2):
                    nc.vector.scalar_tensor_tensor(
                        av8[:, 1, h * HW : (h + 1) * HW], av1h[h], AV_SCALE,
                        B_sb[:, h, :], ALU.mult, ALU.mult)

            def emit_phase2(ch, sm, av8, half, last=False):
                """proj MMs into the sm banks + single res stt + DMA out."""
                ssl = slice(half * HW, (half + 1) * HW)
                isl = slice(ch * IC + half * HW, ch * IC + (half + 1) * HW)
                ps_o = sm  # [P, 2(mt), HW] view of the sm tile
                for mt in range(CT):
                    nc.tensor.matmul(ps_o[:, mt, :],
                                     wo8[:, :, mt * P : (mt + 1) * P],
                                     av8[:, :, ssl], start=True, stop=True,
                                     perf_mode=DR)
                res = resp.tile([P, CT, HW], F32, tag="res")
                nc.vector.scalar_tensor_tensor(res, ps_o, RES_SCALE,
                                               xq[:, :, isl], ALU.mult, ALU.add)
                nc.sync.dma_start(out3[:, :, isl], res)

            for ch in range(NCHK):
                last = ch == NCHK - 1
                carried = "avct1" in state
                # j's where ACT takes both exp halves: load balance (DVE also
                # carries recip/av8) and, on the last chunk, DVE tail drain
                if last:
                    act_h0 = {(2, 1), (5, 1), (8, 1), (15, 0), (15, 1)}
                else:
                    act_h0 = {(2, 1), (5, 1), (8, 1), (11, 1), (14, 1)}

                # sm and av0 are allocated LAZILY: pool-slot WAR deps only
                # cover already-emitted readers, so the alloc must come after
                # the previous chunk's users of the slot are all emitted.
                hold = {"sm": None, "av0": None}

                def get_sm():
                    if hold["sm"] is None:
                        hold["sm"] = psum.tile([P, 2, HW], F32, tag="sm",
                                               name="sm")
                    return hold["sm"]

                def get_av0():
                    if hold["av0"] is None:
                        t = psum.tile([P, IC], F32, tag="av", name="av0")
                        v = t.rearrange("p (a b) -> p a b", a=2)
                        hold["av0"] = [v[:, 0, :], v[:, 1, :]]
                    return hold["av0"]

                exs = []
                av_done = 0
                for jp in range(JPC):
                    ex = emit_scores_pair(ch, jp, act_h0)
                    exs.append(ex)
                    # carried tail work from the previous chunk
                    if jp <= 3 and carried:
                        pav1, pexs = state["avct1"]
                        for q in range(4 * jp, 4 * (jp + 1)):
                            emit_av(pav1, 1, q, pexs[q])
                        if jp == 3:
                            emit_av8_ct1(pav1, *state["p1"])
                            del state["avct1"]
                    if jp in (4, 5) and "p2" in state:
                        pch, psm, pav8 = state["p2"]
                        emit_phase2(pch, psm, pav8, jp - 4)
                        if jp == 5:
                            del state["p2"]
                    # av ct0, one pair behind; deferred past the carried work
                    # (av0 must not be allocated before the previous chunk's
                    # av-ct1 consumers are emitted)
                    if not (carried and jp < 4):
                        while av_done < jp:
                            emit_av(get_av0(), 0, av_done, exs[av_done])
                            av_done += 1
                    if jp == 7:
                        emit_sums(get_sm(), exs, 0, 6, False)
                    if jp == 13:
                        emit_sums(get_sm(), exs, 6, 12, False)
                while av_done < JPC:
                    emit_av(get_av0(), 0, av_done, exs[av_done])
                    av_done += 1
                emit_sums(get_sm(), exs, 12, JPC, True)

                if not last:
                    B_sb, av8 = emit_phase1(ch, get_sm(), get_av0())
                    av1 = psum.tile([P, IC], F32, tag="av", name="av1")
                    av1v = av1.rearrange("p (a b) -> p a b", a=2)
                    state["avct1"] = ([av1v[:, 0, :], av1v[:, 1, :]], exs)
                    state["p1"] = (B_sb, av8)
                    state["p2"] = (ch, get_sm(), av8)
                else:
                    # last chunk: av ct1 accumulates in two scores slots so it
                    # overlaps the recip/B_sb chain instead of serializing
                    # behind av8 ct0's read of the av slot.
                    av1h = [psum.tile([P, HW], F32, tag="sc", name="av1h",
                                      bufs=4) for _ in range(2)]
                    for q in range(JPC):
                        emit_av(av1h, 1, q, exs[q])
                    B_sb, av8 = emit_phase1(ch, get_sm(), get_av0(),
                                            emit_ct0=False)
                    # tail pipelined per i-half: av8(h) -> proj/res/DMA(h)
                    for half in range(2):
                        emit_av8_h(av8, 0, half, get_av0()[half], B_sb)
                        emit_av8_h(av8, 1, half, av1h[half], B_sb)
                        emit_phase2(ch, get_sm(), av8, half, last=True)

        if reps > 1:
            with tc.For_i(0, reps, 1):
                emit_body()
        else:
            emit_body()

    if compile:
        nc.compile()
    return nc


def make_host_inputs(x, gn_w, gn_b, w_qkv, b_qkv, w_out, b_out):
    """Shared (weight) arrays + per-core (x8q, x8o, xq) shards."""
    wqkvT = np.ascontiguousarray(w_qkv.T).astype(ml_dtypes.bfloat16)
    bqkv6 = np.ascontiguousarray(b_qkv.astype(np.float32).reshape(6, P).T)
    woutT = np.ascontiguousarray(w_out.T).astype(np.float32)
    wo8 = (woutT * WO_SCALE).astype(E4NP)
    bout2 = np.ascontiguousarray(b_out.astype(np.float32).reshape(CT, P).T)
    gnw2 = np.ascontiguousarray(gn_w.astype(np.float32).reshape(CT, P).T)
    gnb2 = np.ascontiguousarray(gn_b.astype(np.float32).reshape(CT, P).T)
    pidx = np.arange(P)
    sel = (pidx[:, None] // 32 == np.arange(4)[None, :]).astype(np.float32)
    selT = np.ascontiguousarray(sel.T)

    ones_row = np.ones((1, P), np.float32)
    shared = dict(wqkvT=wqkvT, bqkv6=bqkv6, woutT=woutT, wo8=wo8, bout2=bout2,
                  gnw2=gnw2, gnb2=gnb2, sel=sel, selT=selT, ones_row=ones_row)

    in_maps = []
    for core in range(N_CORES):
        b, h = divmod(core, 2)
        own = slice(h * LH, (h + 1) * LH)
        oth = slice((1 - h) * LH, (2 - h) * LH)
        m = dict(shared)
        xq_f32 = np.ascontiguousarray(x[b][:, own]).astype(np.float32)
        xo_f32 = np.ascontiguousarray(x[b][:, oth]).astype(np.float32)
        m["xq"] = xq_f32
        m["x8q"] = xq_f32.astype(E4NP)
        m["x8o"] = xo_f32.astype(E4NP)
        in_maps.append(m)
    return in_maps


_NC = None


def kernel(x, gn_w, gn_b, w_qkv, b_qkv, w_out, b_out, _trace=False, **_kw):
    global _NC
    x = np.asarray(x)
    if _NC is None:
        _NC = build_nc()
    in_maps = make_host_inputs(np.asarray(x), np.asarray(gn_w), np.asarray(gn_b),
                               np.asarray(w_qkv), np.asarray(b_qkv),
                               np.asarray(w_out), np.asarray(b_out))
    kw = {}
    if _trace:
        kw = dict(trace=True)
    br = run_bass_kernel_spmd(_NC, in_maps, list(range(N_CORES)), **kw)
    out = np.empty((B, C, L), np.float32)
    for core in range(N_CORES):
        b, h = divmod(core, 2)
        out[b][:, h * LH : (h + 1) * LH] = br.results[core]["out"]
    if _trace:
        return out, br
    return out
